# revision 10
# baseline (speedup 1.0000x reference)
"""Causal depthwise temporal conv (K=4) on 8 TRN2 NeuronCores.

Reference semantics (for x: [B, T, D], w: [K, D], b: [D]):
    out[bt, t, d] = sum_{j=0}^{K-1} x_pad[bt, t + j, d] * w[j, d] + b[d]
where x_pad is x left-padded with K-1 zeros along time.

Strategy (v7, fp16 + PE accumulation + balanced engines + big DMAs):
  - Tensor-parallel over channels: core m owns channels [m*512, (m+1)*512).
  - All HBM traffic in fp16 (harness gate is rel_err < 2e-2; fp16 keeps
    max-rel error ~1e-3): halves DMA bytes vs f32.
  - Taps {0,1,3} run on the tensor engine as diagonal-matrix matmuls
    accumulating for free in PSUM (adds on DVE are the scarce resource:
    tensor_tensor is 2x-mode at best, scalar_tensor_tensor is 1x-only).
    ACT evacuates PSUM -> SBUF fp16 fused with the bias add. DVE does
    tap 2 (tensor_scalar, 4x mode) and one tensor_tensor combine (2x).
    On REBAL of every 4 halves, tap 0 moves PE -> DVE (extra TS + TT) to
    equalize the two engines (~78us each).
  - PSUM ping-pong: 2048-col halves, 4 banks each, bufs=2 = all 8 banks.
  - DMA: one HWDGE ring (sync) carries per-chain 1MB loads (8192B rows,
    no runt packets; a FIFO ring serializes per-transfer completion
    latency, so bigger transfers sustain ~270 vs ~210 GB/s), gpsimd SWDGE
    queue carries per-chain 1MB stores; the scalar ring only loads
    weights, so the ACT engine is free to drain PSUM. First chain loads
    in 512-col pieces (PE starts ~2us sooner); last half drains in
    512-col pieces (short tail).
"""

import numpy as np

import concourse.bacc as bacc
import concourse.mybir as mybir
from concourse.tile import TileContext
from concourse import bass_utils

B = 4            # batch
T = 4096         # sequence length
D = 4096         # channels (width)
K = 4            # temporal taps
N_CORES = 8
D_SH = D // N_CORES          # 512 channels per core
P = 128                      # SBUF partitions
N_BLK = D_SH // P            # 4 channel blocks per core
TPP = 4104                   # padded time length in DRAM (4096 + 8)
W_STRIDE = K + 1             # per-blk slot in the f32 wb tile

PE_TAPS = (0, 1, 3)          # taps done as diag-matmuls into PSUM (default)
DVE_TAP = 2                  # tap done on DVE (even shift: 4B alignment)
HALF = 2048                  # psum half-chain width (4 banks)
NB = 512                     # matmul moving-block width (1 psum bank)
REBAL = 4                    # every REBAL-th half moves tap 0 to DVE


def _build(b=B, t=T, n_blk=N_BLK):
    nc = bacc.Bacc("TRN2")
    f16 = mybir.dt.float16
    f32 = mybir.dt.float32
    npe = len(PE_TAPS)
    x = nc.dram_tensor("x", [n_blk, b, P, TPP], f16, kind="ExternalInput")
    wd = nc.dram_tensor("wd", [P, n_blk * npe * P], f16, kind="ExternalInput")
    wb = nc.dram_tensor("wb", [P, n_blk * W_STRIDE], f32, kind="ExternalInput")
    out = nc.dram_tensor("out", [n_blk, b, P, t], f16, kind="ExternalOutput")
    mult, add = mybir.AluOpType.mult, mybir.AluOpType.add
    ident = mybir.ActivationFunctionType.Identity
    nhalf = t // HALF

    with TileContext(nc) as tc:
        with tc.tile_pool(name="px", bufs=3) as px, \
             tc.tile_pool(name="ps", bufs=4) as ps, \
             tc.tile_pool(name="po", bufs=3) as po, \
             tc.tile_pool(name="pw", bufs=1) as pw, \
             tc.tile_pool(name="pp", bufs=2, space="PSUM") as pp:
            wdt = pw.tile([P, n_blk * npe * P], f16, tag="wd")
            nc.scalar.dma_start(wdt[:], wd[:, :])
            wt = pw.tile([P, n_blk * W_STRIDE], f32, tag="wb")
            nc.scalar.dma_start(wt[:], wb[:, :])

            hidx = 0
            for blk in range(n_blk):
                def w(j, blk=blk):
                    return wt[:, blk * W_STRIDE + j:blk * W_STRIDE + j + 1]

                def wdiag(ti, blk=blk):
                    o = (blk * npe + ti) * P
                    return wdt[:, o:o + P]

                for bb in range(b):
                    first = blk == 0 and bb == 0
                    last = blk == n_blk - 1 and bb == b - 1
                    # Per-chain load: 1MB main (8192B rows, runt-free) +
                    # tiny 16B-row tail. First chain lands in 512-col
                    # pieces so the PE starts sooner.
                    X = px.tile([P, TPP], f16, tag="x")
                    if first:
                        for c in range(T // NB):
                            nc.sync.dma_start(
                                X[:, c * NB:(c + 1) * NB],
                                x[blk, bb, :, c * NB:(c + 1) * NB])
                    else:
                        nc.sync.dma_start(X[:, 0:t], x[blk, bb, :, 0:t])
                    nc.sync.dma_start(X[:, t:TPP], x[blk, bb, :, t:TPP])
                    o = po.tile([P, t], f16, tag="o")
                    for h in range(nhalf):
                        base = h * HALF
                        hidx += 1
                        # Every REBAL-th half shifts tap 0 PE -> DVE.
                        dve_t0 = hidx % REBAL == 0
                        taps = PE_TAPS[1:] if dve_t0 else PE_TAPS
                        pt = pp.tile([P, HALF], f32, tag="ps")
                        # PE: psum[c] += sum_{j in taps} diag(w_j) @ x_j
                        # (tap-major; the first chain block-major to chase
                        # its piecewise load)
                        nt = len(taps)
                        order = [(c, ti) for c in range(HALF // NB)
                                 for ti in range(nt)] if first and h == 0 \
                            else [(c, ti) for ti in range(nt)
                                  for c in range(HALF // NB)]
                        for c, ti in order:
                            lo = base + c * NB + taps[ti]
                            nc.tensor.matmul(
                                pt[:, c * NB:(c + 1) * NB],
                                wdiag(PE_TAPS.index(taps[ti])),
                                X[:, lo:lo + NB],
                                start=(ti == 0),
                                stop=(ti == nt - 1),
                                skip_group_check=True,
                            )
                        # Evac + combine; the last half in 512-col pieces.
                        pieces = 4 if (last and h == nhalf - 1) else 1
                        pw_ = HALF // pieces
                        for q in range(pieces):
                            qs = base + q * pw_
                            s = ps.tile([P, pw_], f16, tag="s")
                            nc.scalar.activation(s[:], pt[:, qs - base:
                                                          qs - base + pw_],
                                                 ident, bias=w(K), scale=1.0)
                            y = ps.tile([P, pw_], f16, tag="y")
                            nc.vector.tensor_scalar_mul(
                                y[:], X[:, qs + DVE_TAP:qs + DVE_TAP + pw_],
                                w(DVE_TAP))
                            if dve_t0:
                                y2 = ps.tile([P, pw_], f16, tag="y")
                                nc.vector.scalar_tensor_tensor(
                                    y2[:], X[:, qs:qs + pw_], w(0),
                                    y[:], mult, add)
                                y = y2
                            nc.vector.tensor_tensor(o[:, qs:qs + pw_],
                                                    y[:], s[:], add)
                            if last and h == nhalf - 1:
                                nc.gpsimd.dma_start(
                                    out[blk, bb, :, qs:qs + pw_],
                                    o[:, qs:qs + pw_])
                        # Last chain: store each half as soon as it is
                        # ready (the final one went out in pieces above).
                        if last and h < nhalf - 1:
                            nc.gpsimd.dma_start(
                                out[blk, bb, :, base:base + HALF],
                                o[:, base:base + HALF])
                    # Per-chain 1MB store on the gpsimd SWDGE queue
                    # (runt-free 8192B rows).
                    if not last:
                        nc.gpsimd.dma_start(out[blk, bb, :, :], o[:, :])
    nc.compile()
    return nc


def _prepare(x, w, b):
    x = np.asarray(x, dtype=np.float32)
    w = np.asarray(w, dtype=np.float32)
    b = np.asarray(b, dtype=np.float32)
    npe = len(PE_TAPS)
    # channel-major, left zero-padded time, fp16: [D, B, TPP]
    xp = np.zeros((D, B, TPP), dtype=np.float16)
    xp[:, :, K - 1:K - 1 + T] = x.transpose(2, 0, 1)
    wbt = np.concatenate([w.T, b[:, None]], axis=1).astype(np.float32)  # [D, 5]
    in_maps = []
    for m in range(N_CORES):
        sl = slice(m * D_SH, (m + 1) * D_SH)
        wbm = wbt[sl].reshape(N_BLK, P, W_STRIDE).transpose(1, 0, 2)
        # diag stationary matrices for the PE taps: [P, n_blk*npe*P]
        wdm = np.zeros((P, N_BLK, npe, P), dtype=np.float16)
        rng = np.arange(P)
        for blk in range(N_BLK):
            for ti, tap in enumerate(PE_TAPS):
                wdm[rng, blk, ti, rng] = w[tap, m * D_SH + blk * P + rng]
        in_maps.append({
            "x": np.ascontiguousarray(
                xp[sl].reshape(N_BLK, P, B, TPP).transpose(0, 2, 1, 3)),
            "wd": np.ascontiguousarray(wdm).reshape(P, N_BLK * npe * P),
            "wb": np.ascontiguousarray(wbm).reshape(P, N_BLK * W_STRIDE),
        })
    return in_maps


def _collect(results):
    out = np.empty((B, T, D), dtype=np.float32)
    for m in range(N_CORES):
        o = np.asarray(results[m]["out"]).astype(np.float32)
        o = o.reshape(N_BLK, B, P, T).transpose(1, 3, 0, 2).reshape(B, T, D_SH)
        out[:, :, m * D_SH:(m + 1) * D_SH] = o
    return out


def _run(in_maps, trace=False, **kwargs):
    nc = _build()
    return bass_utils.run_bass_kernel_spmd(
        nc, in_maps, core_ids=list(range(N_CORES)), trace=trace, **kwargs)


def kernel(x, w, b):
    in_maps = _prepare(x, w, b)
    try:
        res = _run(in_maps)
    except Exception:
        # Transient NRT device errors have been observed on a cold first
        # execute; one retry (fresh compile dir) clears them.
        res = _run(in_maps)
    return _collect(res.results)
